# revision 26
# baseline (speedup 1.0000x reference)
"""CTC loss (nn_CTC_28819230556189) on 8 Trainium2 NeuronCores via Bass/Tile.

Strategy (data-parallel over batch, 4 examples per core):
  - logits = hpad @ W.T (b == 0) ; log-softmax over V=5000 ; CTC forward DP.
  - Per (b, t) only two reductions of the V-row are needed:
      Z[b,t]    = sum_v exp(logits - C)            (constant-shift logsumexp)
      glog[b,t,s] = logits[b, t, ext[b,s]]         (host-gathered W[ext] matmul)
  - Main matmul + glog matmul run in fp8(e4m3) DoubleRow mode: 256-deep
    contraction per instruction at 0.5 cycles/row (2x bf16 throughput).
  - Softmax denominator without Ln: rinv = 1/Z on DVE (reciprocal), then
    p = exp(glog - C + SC) * rinv via tensor_scalar mul (DVE 4x mode).
  - DP in the linear domain, alpha renormalized every 16 steps:
      alpha'[s] = (alpha[s] + alpha[s-1] + allow2[s]*alpha[s-2]) * p[s]
    done in 3 DVE ops per step using tensor_tensor_scan: with the CTC
    interleaved mask (allow2 == 0 on every even/blank lane) the scan
      state[s] = allow2[s]*state[s-1] + alpha[s-1]
    truncates exactly to alpha[s-1] + allow2[s]*alpha[s-2].
    Lanes are trimmed to the reachable range [s_lo(t), s_hi(t)).
  - loss partials summed on host (no collectives needed).
"""

import numpy as np
import ml_dtypes

import concourse.bass as bass
import concourse.bacc as bacc
import concourse.tile as tile
import concourse.mybir as mybir
from concourse.bass_utils import run_bass_kernel_spmd

BF16 = mybir.dt.bfloat16
F8 = mybir.dt.float8e4
F32 = mybir.dt.float32
AF = mybir.ActivationFunctionType
ALU = mybir.AluOpType
AX = mybir.AxisListType
PERF = mybir.MatmulPerfMode.DoubleRow

# Problem shapes (hardcoded per spec nn_CTC_28819230556189)
B, T, E, V, L = 32, 500, 1024, 5000, 100
S = 2 * L + 1          # 201 extended labels
NCORE = 8
BPC = B // NCORE       # 4 examples per core
KT = E // 256          # 4 fp8 DoubleRow contraction super-tiles
VC = 500               # v-chunk width (one PSUM bank in f32)
NV = V // VC           # 10
TCP = 128              # time-chunk partition size (DoubleRow needs M=128)
NCHUNK = 4             # chunks cover T=500 as [128, 128, 128, 116] (T padded
                       # to 512 on host; pad rows are neutralized exactly)
TPAD = TCP * NCHUNK    # 512
C_SHIFT = 4.0          # logsumexp constant shift (logits ~ N(0,1))
SC = 2.9               # per-step scale: p_raw = exp(glog - C + SC), unnormalized;
                       # sum_t ln(Z) is subtracted in the finalize instead.
                       # 2.9 makes the expected per-step alpha drift ~zero.
RENORM = 32            # renormalize alpha every RENORM steps
NR = (T - 1) // RENORM + 1  # renorm slots (32, 0..31 used)
SP2 = 256              # p row padded to 256 cols (512B) for full-rate DMA

_cache = {}


def _rng(t):
    """Active lane range [lo, hi) of the CTC lattice at time t."""
    hi = min(S, 2 * t + 2)
    lo = max(0, S - 2 * (T - t))
    return lo, hi


def _build_nc():
    nc = bacc.Bacc("TRN2", target_bir_lowering=False, debug=False,
                   enable_asserts=False)

    # register const APs used as activation biases
    for val in (-C_SHIFT, SC - C_SHIFT):
        _cth = nc.alloc_sbuf_tensor(f"const-f32-{val}", [128, 1], F32)
        nc.gpsimd.memset(_cth.ap(), val)
        nc.const_aps.aps[(F32, val)] = _cth.ap()
    nc.all_engine_barrier()

    hp_d = nc.dram_tensor("hp8", [128, BPC, KT, NCHUNK, 2, TCP], F8, kind="ExternalInput")
    w_d = nc.dram_tensor("w8", [NV, 128, KT, 2, VC], F8, kind="ExternalInput")
    wx_d = nc.dram_tensor("wx8", [128, KT, BPC, 2, SP2], F8, kind="ExternalInput")
    m2_d = nc.dram_tensor("m2", [BPC, S], BF16, kind="ExternalInput")
    out_d = nc.dram_tensor("out", [1, 1], F32, kind="ExternalOutput")

    with tile.TileContext(nc) as tc:
      with tc.tile_pool(name="persist", bufs=1) as pers:
        def ptile(shape, dtype, nm):
            return pers.tile(shape, dtype, tag=nm, name=nm)

        # ---- resident inputs ----
        # Ordered so the first (bb0, v0) matmul + glog can start ASAP.
        w_all = ptile([128, NV, KT, 2, VC], F8, "w_all")
        hp_all = ptile([128, BPC, KT, NCHUNK, 2, TCP], F8, "hp_all")
        wx_all = ptile([128, KT, BPC, 2, SP2], F8, "wx_all")
        m2t = ptile([BPC, S], BF16, "m2t")
        # Loads split across the SP and gpsimd queues so they overlap; the
        # glog inputs (hp, wx) go first — they gate the p-probability path.
        nc.sync.dma_start(hp_all[:, 0], hp_d[:, 0])
        nc.gpsimd.dma_start(hp_all[:, 1], hp_d[:, 1])
        nc.gpsimd.dma_start(hp_all[:, 2], hp_d[:, 2])
        nc.gpsimd.dma_start(hp_all[:, 3], hp_d[:, 3])
        nc.sync.dma_start(wx_all[:], wx_d[:])
        nc.sync.dma_start(m2t[:], m2_d[:])
        for v in range(0, NV, 2):
            nc.sync.dma_start(w_all[:, v], w_d[v])
        for v in range(1, NV, 2):
            nc.gpsimd.dma_start(w_all[:, v], w_d[v])

        # ---- DP state (persistent) ----
        # alpha[s] lives at col s+2; cols 0,1 are a zero halo for the shifts.
        A0 = ptile([BPC, S + 2], BF16, "A0")
        A1 = ptile([BPC, S + 2], BF16, "A1")
        nc.vector.memset(A0[:], 0.0)
        nc.vector.memset(A1[:], 0.0)
        t1 = ptile([BPC, S], BF16, "t1")
        t3 = ptile([BPC, S], BF16, "t3")
        R = ptile([BPC, NR], F32, "R")
        nc.vector.memset(R[:], 1.0)
        rinv = ptile([BPC, 1], F32, "rinv")
        spart_all = ptile([TCP, BPC, NCHUNK, NV], F32, "spart_all")
        ones = ptile([TCP, 1], F32, "ones")
        nc.vector.memset(ones[:], 1.0)

        with (
            tc.tile_pool(name="ps", bufs=3, space="PSUM") as ps_pool,
            tc.tile_pool(name="glog", bufs=2, space="PSUM") as glog_pool,
            tc.tile_pool(name="scr", bufs=2) as scr_pool,
            tc.tile_pool(name="small", bufs=4) as small_pool,
            tc.tile_pool(name="pg", bufs=2) as pg_pool,
            tc.tile_pool(name="pb", bufs=2) as pb_pool,
        ):
            cur = A0
            for c in range(NCHUNK):
                t0 = c * TCP
                tcn = min(TCP, T - t0)  # real timesteps in this chunk
                PB = pb_pool.tile([BPC, TCP * SP2], BF16, tag="pb")
                pg = pg_pool.tile([TCP, BPC, SP2], BF16, tag="pg")
                # p-probability path first: glog matmul + exp per bb, then
                # the transpose-flatten DMAs. The V-vocabulary exp work only
                # feeds the finalize (sum_t ln Z) and is emitted after, so
                # it never gates the DP.
                for bb in range(BPC):
                    glog = glog_pool.tile([TCP, SP2], F32, tag="glog")
                    for kt in range(KT):
                        nc.tensor.matmul(
                            glog[:],
                            hp_all[:, bb, kt, c],
                            wx_all[:, kt, bb],
                            start=(kt == 0), stop=(kt == KT - 1),
                            perf_mode=PERF)
                    nc.scalar.activation(pg[:, bb, :], glog[:], AF.Exp,
                                         bias=SC - C_SHIFT, scale=1.0)
                # transpose-flatten [TC, 512B] -> PB row bb [1, TC*SP2],
                # split into time-eighths, interleaved across the two bb of
                # each queue, so the DP can start on the first slice while
                # later slices still transfer. bb0/bb2 ride the SP queue,
                # bb1/bb3 the gpsimd queue; the queues' transfers overlap.
                es = [0, 16, 32, 48, 64, 80, 96, 112, TCP]
                for e in range(8):
                    a, z = es[e], es[e + 1]
                    for bb in range(BPC):
                        eng = nc.sync if bb % 2 == 0 else nc.gpsimd
                        eng.dma_start(PB[bb:bb + 1, a * SP2:z * SP2],
                                      pg[a:z, bb, :])
                for bb in range(BPC):
                    for v in range(NV):
                        ps = ps_pool.tile([TCP, VC], F32, tag="ps")
                        for kt in range(KT):
                            nc.tensor.matmul(
                                ps[:],
                                hp_all[:, bb, kt, c],
                                w_all[:, v, kt],
                                start=(kt == 0), stop=(kt == KT - 1),
                                perf_mode=PERF)
                        scr = scr_pool.tile([TCP, VC], BF16, tag="scr")
                        nc.scalar.activation(scr[:], ps[:], AF.Exp,
                                             bias=-C_SHIFT, scale=1.0,
                                             accum_out=spart_all[:, bb, c, v:v + 1])

                # ---- DP steps for this chunk ----
                for tl in range(tcn):
                    t = t0 + tl
                    lo, hi = _rng(t)
                    pc = PB[:, tl * SP2:tl * SP2 + S]
                    if t == 0:
                        nc.vector.tensor_copy(cur[:, 2:4], pc[:, 0:2])
                        continue
                    prv, cur = cur, (A1 if cur is A0 else A0)
                    # t3[s] = alpha[s-1] + m2[s]*alpha[s-2] for s in [lo, hi)
                    nc.vector.tensor_tensor_scan(
                        t3[:, lo:hi], m2t[:, lo:hi], prv[:, lo + 1:hi + 1],
                        prv[:, lo:lo + 1], op0=ALU.mult, op1=ALU.add)
                    nc.vector.tensor_add(t1[:, lo:hi], prv[:, lo + 2:hi + 2],
                                         t3[:, lo:hi])
                    nc.vector.tensor_mul(cur[:, lo + 2:hi + 2], t1[:, lo:hi],
                                         pc[:, lo:hi])
                    if t % RENORM == RENORM - 1:
                        k = t // RENORM
                        nc.vector.tensor_reduce(R[:, k:k + 1],
                                                cur[:, lo + 2:hi + 2],
                                                axis=AX.X, op=ALU.add)
                        nc.vector.reciprocal(rinv[:], R[:, k:k + 1])
                        nc.vector.tensor_scalar_mul(cur[:, lo + 2:hi + 2],
                                                    cur[:, lo + 2:hi + 2],
                                                    rinv[:])

            # ---- finalize ----
            # ll = ln(a[-1]+a[-2]) + sum_k ln(R_k) - sum_t ln(lsum_t) - T*SC
            # (the -T*SC part is applied on the host)
            u = ptile([BPC, 1], F32, "u")
            nc.vector.tensor_add(u[:], cur[:, S:S + 1], cur[:, S + 1:S + 2])
            lnu = ptile([BPC, 1], F32, "lnu")
            nc.scalar.activation(lnu[:], u[:], AF.Ln)
            rlog = ptile([BPC, NR], F32, "rlog")
            nc.scalar.activation(rlog[:], R[:], AF.Ln)
            # sum_t ln(Z) entirely on Act (accumulators) + PE so no DVE op
            # here can head-block queued DP work while waiting on Act.
            lsums = ptile([TCP, BPC, NCHUNK], F32, "lsums")
            jid = ptile([TCP, NV], BF16, "jid")
            jln = ptile([TCP, NCHUNK], F32, "jln")
            zsum = ptile([TCP, BPC], F32, "zsum")
            TL = T - (NCHUNK - 1) * TCP  # real rows in the last chunk (116)
            for bb in range(BPC):
                for c in range(NCHUNK):
                    nc.scalar.activation(jid[:], spart_all[:, bb, c, :],
                                         AF.Identity,
                                         accum_out=lsums[:, bb, c:c + 1])
            # ln(Z) sums: full rows for chunks 0..2, rows [0:TL) for the
            # padded last chunk (its pad rows hold garbage).
            zsumB = ptile([TCP, BPC], F32, "zsumB")
            for bb in range(BPC):
                nc.scalar.activation(jln[:, 0:NCHUNK - 1],
                                     lsums[:, bb, 0:NCHUNK - 1], AF.Ln,
                                     accum_out=zsum[:, bb:bb + 1])
                nc.scalar.activation(jln[0:TL, NCHUNK - 1:NCHUNK],
                                     lsums[0:TL, bb, NCHUNK - 1:NCHUNK], AF.Ln,
                                     accum_out=zsumB[0:TL, bb:bb + 1])
            rs = ptile([BPC, 1], F32, "rs")
            nc.vector.tensor_reduce(rs[:], rlog[:], axis=AX.X, op=ALU.add)
            llv = ptile([BPC, 1], F32, "llv")
            nc.vector.tensor_add(llv[:], lnu[:], rs[:])
            llf = ptile([1, BPC], F32, "llf")
            nc.sync.dma_start(llf[:], llv[:])  # [4,1] -> [1,4] partition flatten
            with tc.tile_pool(name="zp", bufs=1, space="PSUM") as zp_pool:
                zps = zp_pool.tile([1, BPC], F32, tag="zps")
                nc.tensor.matmul(zps[:], ones[:], zsum[:], start=True, stop=False)
                nc.tensor.matmul(zps[:], ones[0:TL], zsumB[0:TL], start=False,
                                 stop=True)
                lltot = ptile([1, BPC], F32, "lltot")
                nc.vector.tensor_sub(lltot[:], llf[:], zps[:])
            tot = ptile([1, 1], F32, "tot")
            nc.vector.tensor_reduce(tot[:], lltot[:], axis=AX.X, op=ALU.add)
            nc.sync.dma_start(out_d[:], tot[:])

    nc.compile()
    return nc


def prep_inputs(hpad, W, ys):
    """Host prep: fp8 casts + layout shuffles. Returns per-core in_maps."""
    F8NP = ml_dtypes.float8_e4m3fn
    ext = np.zeros((B, S), dtype=np.int64)
    ext[:, 1::2] = ys
    prev2 = np.full((B, S), -1, dtype=np.int64)
    prev2[:, 2:] = ext[:, :-2]
    allow2 = ((ext != 0) & (ext != prev2)).astype(ml_dtypes.bfloat16)

    # hp8 [128, B, KT, NCHUNK, 2, TCP]:
    #   element (p, b, kt, c, i, tau) = hpad[b, c*TCP+tau, kt*256+p*2+i]
    # (t >= T is zero padding; those DP steps are skipped and their ln(Z)
    #  contribution is forced to 0 on device)
    hpp = np.zeros((B, E, TPAD), dtype=np.float32)
    hpp[:, :, :T] = np.asarray(hpad).transpose(0, 2, 1)
    hp8 = np.ascontiguousarray(
        hpp.reshape(B, KT, 128, 2, NCHUNK, TCP).transpose(2, 0, 1, 4, 3, 5)
    ).astype(F8NP)
    # w8 [NV, 128, KT, 2, VC]: element (v, p, kt, i, j) = W[v*VC+j, kt*256+p*2+i]
    w8 = np.ascontiguousarray(
        np.asarray(W).reshape(NV, VC, KT, 128, 2).transpose(0, 3, 2, 4, 1)
    ).astype(F8NP)
    # wx8 [128, KT, B, 2, SP2]: element (p, kt, b, i, s) = W[ext[b,s], kt*256+p*2+i]
    # (s >= S is zero padding so glog/pg are written full width)
    wx8 = np.zeros((128, KT, B, 2, SP2), dtype=F8NP)
    wx8[:, :, :, :, :S] = np.asarray(W)[ext.reshape(-1)].reshape(B, S, KT, 128, 2) \
        .transpose(3, 2, 0, 4, 1).astype(F8NP)

    in_maps = []
    for c in range(NCORE):
        sl = slice(c * BPC, (c + 1) * BPC)
        in_maps.append({
            "hp8": np.ascontiguousarray(hp8[:, sl]),
            "w8": w8,
            "wx8": np.ascontiguousarray(wx8[:, :, sl]),
            "m2": np.ascontiguousarray(allow2[sl]),
        })
    return in_maps


def kernel(hpad, W, b, ys):
    assert hpad.shape == (B, T, E) and W.shape == (V, E) and ys.shape == (B, L)
    assert not np.any(np.asarray(b)), "kernel assumes b == 0 (per problem spec)"

    if "nc" not in _cache:
        _cache["nc"] = _build_nc()
    nc = _cache["nc"]

    in_maps = prep_inputs(hpad, W, ys)
    res = run_bass_kernel_spmd(nc, in_maps, core_ids=list(range(NCORE)))
    tot = sum(float(r["out"][0, 0]) for r in res.results)
    tot -= B * T * SC  # undo the per-step exp(SC) scaling of p
    return np.float32(-tot / B)
